# revision 2
# baseline (speedup 1.0000x reference)
"""Cross-attention kernel for Trainium2 (Bass/Tile), 8 NeuronCores — v2.

Transpose-free formulation: mm1 computes S^T = ref @ dom^T directly
(lhsT = refT chunk, rhs = domT chunk), so the exp output is already P^T
in the [key, query] orientation mm2 needs as lhsT — the 128 per-batch PE
transposes, their PSUM->SBUF copies, and the scalar accumulator reads of
the baseline all disappear.

Softmax row sums (now along the partition axis) come from ones-lhsT
matmuls (lhsT = [128,1] ones, rhs = P^T tile) accumulating into a [1,512]
PSUM row per query half; a reciprocal plus four PE transposes of that row
bridge it back to per-partition [128,1] scale layout for the x eviction.

Per batch b:
    S^T = (ref @ dom^T) * SCALE          [N, N]  (16 psum tiles [128,512])
    P^T = exp(S^T)                       rowsums via ones-lhsT matmuls
    x   = P @ ref                        [N, C]  (lhsT = P^T tiles directly)
    out[2*cp+e, j] = sum_q x[512e+q, cp] wt[q, j] + bias[j]   (scramble+linear
        fused: x tiles are mm3's lhsT in natural layout)

Sharding: data-parallel over batch. B=16 -> 2 batches per core, no
collectives. All matmuls fp32r (full PE rate at free dim >= 512).

DMA: Q0 (gpsimd SWDGE) streams domT/wt/bias + out stores; Q1 (sync HWDGE)
streams eye/refT/ref. Batch-0 first-half loads are k-chunk granular so the
first mm1 accumulation starts on the earliest 512KB.
"""

import os
from contextlib import ExitStack

import numpy as np

import concourse.bass as bass
import concourse.mybir as mybir
import concourse.tile as tile
from concourse import bacc
from concourse._compat import with_exitstack
from concourse.bass_utils import run_bass_kernel_spmd

B, N, C = 16, 1024, 512
NUM_HEADS = 8
SCALE = (C // NUM_HEADS) ** -0.5  # 0.125
CORES = 8
BPC = B // CORES  # batches per core

P = 128          # partitions
NT = N // P      # 8 query tiles
CCH = C // P     # 4 contraction chunks over channels
MH = N // 512    # 2 query halves
MCH = N // P     # 8 key chunks
JT = C // P      # 4 output-column blocks per half

F32 = mybir.dt.float32
F32R = mybir.dt.float32r

USE_F32R = os.environ.get("KERNEL_F32R", "1") == "1"
WARMUP_MMS = int(os.environ.get("KERNEL_WARMUP", "42"))


def _r(ap):
    return ap.bitcast(F32R) if USE_F32R else ap


@with_exitstack
def _core_kernel(ctx: ExitStack, tc: tile.TileContext,
                 domt_d, reft_d, ref_d, wt_d, bias_d, eye_d, out_d):
    nc = tc.nc

    consts = ctx.enter_context(tc.tile_pool(name="consts", bufs=1))
    identity = consts.tile([P, P], F32)
    nc.sync.dma_start(_r(identity[:]), _r(eye_d[:, :]))

    ps_S = ctx.enter_context(tc.tile_pool(name="ps_s", bufs=3, space="PSUM"))
    ps_X = ctx.enter_context(tc.tile_pool(name="ps_x", bufs=3, space="PSUM"))
    ps_R = ctx.enter_context(tc.tile_pool(name="ps_r", bufs=1, space="PSUM"))
    ps_T = ctx.enter_context(tc.tile_pool(name="ps_t", bufs=1, space="PSUM"))

    # PE warmup: dependency-free matmuls on memset zeros while the first
    # input DMAs stream, so the HAM clock gate reaches full rate before
    # real work arrives.
    if WARMUP_MMS:
        zsrc = consts.tile([P, 640], F32)
        nc.vector.memset(zsrc[:], 0.0)
        zr = consts.tile([P, 640], F32)
        nc.vector.tensor_copy(_r(zr[:]), zsrc[:])
        warm_ps = ps_X.tile([P, 512], F32, tag="ps_x")
        for i in range(WARMUP_MMS):
            nc.tensor.matmul(warm_ps[:], _r(zr[:, :P]), _r(zr[:, P:640]),
                             start=True, stop=True)

    p_ref = ctx.enter_context(tc.tile_pool(name="ref", bufs=2))
    p_domT = ctx.enter_context(tc.tile_pool(name="domT", bufs=2))
    p_refT = ctx.enter_context(tc.tile_pool(name="refT", bufs=2))
    p_Pt = ctx.enter_context(tc.tile_pool(name="probsT", bufs=3))
    p_x = ctx.enter_context(tc.tile_pool(name="x", bufs=8))
    p_out = ctx.enter_context(tc.tile_pool(name="out", bufs=4))
    p_stats = ctx.enter_context(tc.tile_pool(name="stats", bufs=4))

    # ---- pre-emit every input DMA so the rings stream continuously ----
    def load_T(sb, dr, b, eng, k_granular=False):
        # [C, N] host-pretransposed tensor: chunk k lands at cols
        # [k*N, (k+1)*N); within a chunk, key/query index is the column.
        for h in range(MH):
            if k_granular and h == 0:
                for k in range(CCH):
                    eng.dma_start(
                        _r(sb[:, k * N + h * 512: k * N + (h + 1) * 512]),
                        _r(dr[b, k * P:(k + 1) * P, h * 512:(h + 1) * 512]),
                    )
            else:
                eng.dma_start(
                    _r(sb[:, :].rearrange("p (k n) -> p k n", k=CCH)
                       [:, :, h * 512:(h + 1) * 512]),
                    _r(dr[b, :, h * 512:(h + 1) * 512]
                       .rearrange("(k p) c -> p k c", p=P)),
                )

    def load_nat(sb, dr, b, eng):
        # tile col block t holds rows [128t, 128(t+1)) of the [N, C] matrix
        eng.dma_start(
            _r(sb[:, :].rearrange("p (t c) -> p t c", t=NT)),
            _r(dr[b].rearrange("(t p) c -> p t c", p=P)),
        )

    domT_sbs = [p_domT.tile([P, CCH * N], F32, tag="domT", name=f"domT_sb{i}")
                for i in range(BPC)]
    refT_sbs = [p_refT.tile([P, CCH * N], F32, tag="refT", name=f"refT_sb{i}")
                for i in range(BPC)]
    ref_sbs = [p_ref.tile([P, NT * C], F32, tag="ref", name=f"ref_sb{i}")
               for i in range(BPC)]

    # Q1 (sync): eye, refT0, ref0, refT1, ref1
    load_T(refT_sbs[0], reft_d, 0, nc.sync, k_granular=True)
    load_nat(ref_sbs[0], ref_d, 0, nc.sync)
    # Q0 (gpsimd): domT0, wt, domT1, bias
    load_T(domT_sbs[0], domt_d, 0, nc.gpsimd, k_granular=True)
    wt_sb = consts.tile([P, CCH * C], F32)
    nc.gpsimd.dma_start(
        _r(wt_sb[:, :].rearrange("p (q c) -> p q c", q=CCH)),
        _r(wt_d.rearrange("(q p) c -> p q c", p=P)),
    )
    if BPC > 1:
        load_T(refT_sbs[1], reft_d, 1, nc.sync)
        load_nat(ref_sbs[1], ref_d, 1, nc.sync)
        load_T(domT_sbs[1], domt_d, 1, nc.gpsimd)
    bias_sb = consts.tile([P, C], F32)
    nc.gpsimd.dma_start(bias_sb[:], bias_d.partition_broadcast(P))

    # ones column for rowsum matmuls; rs staging tile for the recip bridge
    ones_f = consts.tile([P, 1], F32)
    nc.vector.memset(ones_f[:], 1.0)
    ones_r = consts.tile([P, 1], F32)
    nc.vector.tensor_copy(_r(ones_r[:]), ones_f[:])
    rs_sb = consts.tile([P, 512], F32)
    nc.vector.memset(rs_sb[:], 0.0)

    for b in range(BPC):
        domT_sb = domT_sbs[b]
        refT_sb = refT_sbs[b]
        ref_sb = ref_sbs[b]

        out_v = out_d[b].rearrange("(n2 two) j -> two n2 j", two=2)

        Pt_sbs = {}
        ps_rs = {}
        recip_sbs = {}
        x_tiles = []

        def mm1_group(h, mi):
            # S^T tile [key(mi) 128, query 512] -> exp into Pt
            if mi == 0:
                Pt_sbs[h] = p_Pt.tile([P, MCH * 512], F32, tag="probsT",
                                      name=f"Pt_sb{b}_{h}")
                ps_rs[h] = ps_R.tile([1, 512], F32, tag="ps_r",
                                     name=f"ps_r{b}_{h}")
            Pt_sb = Pt_sbs[h]
            ps_s = ps_S.tile([P, 512], F32, tag="ps_s",
                             name=f"ps_s{b}_{h}_{mi}")
            for k in range(CCH):
                nc.tensor.matmul(
                    ps_s[:],
                    _r(refT_sb[:, k * N + mi * P: k * N + (mi + 1) * P]),
                    _r(domT_sb[:, k * N + h * 512: k * N + (h + 1) * 512]),
                    start=(k == 0), stop=(k == CCH - 1),
                )
            nc.scalar.activation(_r(Pt_sb[:, mi * 512:(mi + 1) * 512]),
                                 ps_s[:],
                                 mybir.ActivationFunctionType.Exp,
                                 scale=float(SCALE))

        def rowsum(h, mi):
            # ones-lhsT matmul: [1,512] column sums of the P^T tile,
            # accumulated over mi into ps_r
            nc.tensor.matmul(ps_rs[h][:], _r(ones_r[:]),
                             _r(Pt_sbs[h][:, mi * 512:(mi + 1) * 512]),
                             start=(mi == 0), stop=(mi == MCH - 1))

        def bridge(h):
            # [1,512] rowsums -> SBUF -> 4 PE transposes -> reciprocal on
            # the 4 populated columns -> [128,4] per-partition scales
            nc.vector.tensor_copy(rs_sb[0:1, :], ps_rs.pop(h)[:])
            ps_t = ps_T.tile([P, 512], F32, tag="ps_t", name=f"ps_t{b}_{h}")
            for t in range(4):
                nc.tensor.transpose(ps_t[:, t * P:(t + 1) * P],
                                    rs_sb[:, t * P:(t + 1) * P],
                                    identity[:])
            recip_sb = p_stats.tile([P, 4], F32, tag="recip",
                                    name=f"recip{b}_{h}")
            nc.vector.reciprocal(
                recip_sb[:, :],
                ps_t[:, :].rearrange("p (t c) -> p t c", t=4)[:, :, 0])
            recip_sbs[h] = recip_sb

        def mm2_group(h, nl):
            # x tile [query 128, C] = sum_mi Pt(mi)^T @ ref chunk
            Pt_sb = Pt_sbs[h]
            ps_x = ps_X.tile([P, C], F32, tag="ps_x",
                             name=f"ps_x{b}_{h}_{nl}")
            for mi in range(MCH):
                nc.tensor.matmul(
                    ps_x[:],
                    _r(Pt_sb[:, mi * 512 + nl * P: mi * 512 + (nl + 1) * P]),
                    _r(ref_sb[:, mi * C:(mi + 1) * C]),
                    start=(mi == 0), stop=(mi == MCH - 1),
                )
            return ps_x

        def evict_x(h, nl, ps_x):
            x_t = p_x.tile([P, C], F32, tag="x", name=f"x_t{b}_{h}_{nl}")
            nc.vector.tensor_scalar_mul(_r(x_t[:]), ps_x[:],
                                        recip_sbs[h][:, nl:nl + 1])
            x_tiles.append(x_t)

        def mm3_group(e, cb):
            # out rows (2*cp + e) = x_half_e^T @ wt + bias; evict + store
            # in halves so the final store chain pipelines
            ps_z = ps_X.tile([P, C], F32, tag="ps_x",
                             name=f"ps_z{b}_{e}_{cb}")
            for q in range(CCH):
                x_t = x_tiles[e * CCH + q]
                nc.tensor.matmul(
                    ps_z[:],
                    _r(x_t[:, cb * P:(cb + 1) * P]),
                    _r(wt_sb[:, q * C:(q + 1) * C]),
                    start=(q == 0), stop=(q == CCH - 1),
                )
            o_sb = p_out.tile([P, C], F32, tag="out",
                              name=f"o_sb{b}_{e}_{cb}")
            for piece in range(2):
                sl = slice(piece * (C // 2), (piece + 1) * (C // 2))
                nc.vector.tensor_add(o_sb[:, sl], ps_z[:, sl],
                                     bias_sb[:, sl])
                nc.gpsimd.dma_start(out_v[e, cb * P:(cb + 1) * P, sl],
                                    o_sb[:, sl])

        # mm1 half 0 (rowsums lag two mi groups behind the exp evictions)
        for mi in range(MCH):
            mm1_group(0, mi)
            if mi >= 2:
                rowsum(0, mi - 2)
        rowsum(0, MCH - 2)
        rowsum(0, MCH - 1)
        # mm1 half 1, with half 0's recip bridge tucked after the first
        # group so the PE never waits on the vector engine
        mm1_group(1, 0)
        bridge(0)
        for mi in range(1, MCH):
            mm1_group(1, mi)
            if mi >= 2:
                rowsum(1, mi - 2)
        rowsum(1, MCH - 2)
        rowsum(1, MCH - 1)
        # mm2/mm3 half 0, with half 1's bridge tucked after the first group
        ps_x0 = mm2_group(0, 0)
        bridge(1)
        evict_x(0, 0, ps_x0)
        for nl in range(1, 4):
            evict_x(0, nl, mm2_group(0, nl))
        for cb in range(JT):
            mm3_group(0, cb)
        for nl in range(4):
            evict_x(1, nl, mm2_group(1, nl))
        for cb in range(JT):
            mm3_group(1, cb)


_CACHED = {}


def _build():
    key = ("nc", USE_F32R, WARMUP_MMS)
    if key in _CACHED:
        return _CACHED[key]
    nc = bacc.Bacc("TRN2", target_bir_lowering=False, debug=False)
    domt_d = nc.dram_tensor("domt", [BPC, C, N], F32, kind="ExternalInput").ap()
    reft_d = nc.dram_tensor("reft", [BPC, C, N], F32, kind="ExternalInput").ap()
    ref_d = nc.dram_tensor("ref", [BPC, N, C], F32, kind="ExternalInput").ap()
    wt_d = nc.dram_tensor("wt", [C, C], F32, kind="ExternalInput").ap()
    bias_d = nc.dram_tensor("bias", [C], F32, kind="ExternalInput").ap()
    eye_d = nc.dram_tensor("eye", [P, P], F32, kind="ExternalInput").ap()
    out_d = nc.dram_tensor("out", [BPC, N, C], F32, kind="ExternalOutput").ap()

    with tile.TileContext(nc) as tc:
        _core_kernel(tc, domt_d, reft_d, ref_d, wt_d, bias_d, eye_d, out_d)
    nc.compile()
    _CACHED[key] = nc
    return nc


LAST_RESULTS = None


def kernel(dom, ref, proj_w, proj_b):
    global LAST_RESULTS
    dom = np.ascontiguousarray(np.asarray(dom, dtype=np.float32))
    ref = np.ascontiguousarray(np.asarray(ref, dtype=np.float32))
    wt = np.ascontiguousarray(np.asarray(proj_w, dtype=np.float32).T)
    bias = np.ascontiguousarray(np.asarray(proj_b, dtype=np.float32))
    eye = np.eye(P, dtype=np.float32)

    domt = np.ascontiguousarray(dom.transpose(0, 2, 1))
    reft = np.ascontiguousarray(ref.transpose(0, 2, 1))
    nc = _build()
    in_maps = [
        {
            "domt": domt[c * BPC:(c + 1) * BPC],
            "reft": reft[c * BPC:(c + 1) * BPC],
            "ref": ref[c * BPC:(c + 1) * BPC],
            "wt": wt,
            "bias": bias,
            "eye": eye,
        }
        for c in range(CORES)
    ]
    res = run_bass_kernel_spmd(nc, in_maps, list(range(CORES)))
    LAST_RESULTS = res
    if res.exec_time_ns is not None:
        print(f"HW exec time: {res.exec_time_ns} ns")
    return np.concatenate([r["out"] for r in res.results], axis=0)


# revision 3
# speedup vs baseline: 1.0963x; 1.0963x over previous
"""Cross-attention kernel for Trainium2 (Bass/Tile), 8 NeuronCores — v2.

Transpose-free formulation: mm1 computes S^T = ref @ dom^T directly
(lhsT = refT chunk, rhs = domT chunk), so the exp output is already P^T
in the [key, query] orientation mm2 needs as lhsT — the 128 per-batch PE
transposes, their PSUM->SBUF copies, and the scalar accumulator reads of
the baseline all disappear.

Softmax row sums (now along the partition axis) come from ones-lhsT
matmuls (lhsT = [128,1] ones, rhs = P^T tile) accumulating into a [1,512]
PSUM row per query half; a reciprocal plus four PE transposes of that row
bridge it back to per-partition [128,1] scale layout for the x eviction.

Per batch b:
    S^T = (ref @ dom^T) * SCALE          [N, N]  (16 psum tiles [128,512])
    P^T = exp(S^T)                       rowsums via ones-lhsT matmuls
    x   = P @ ref                        [N, C]  (lhsT = P^T tiles directly)
    out[2*cp+e, j] = sum_q x[512e+q, cp] wt[q, j] + bias[j]   (scramble+linear
        fused: x tiles are mm3's lhsT in natural layout)

Sharding: data-parallel over batch. B=16 -> 2 batches per core, no
collectives. All matmuls fp32r (full PE rate at free dim >= 512).

DMA: Q0 (gpsimd SWDGE) streams domT/wt/bias + out stores; Q1 (sync HWDGE)
streams eye/refT/ref. Batch-0 first-half loads are k-chunk granular so the
first mm1 accumulation starts on the earliest 512KB.
"""

import os
from contextlib import ExitStack

import numpy as np

import concourse.bass as bass
import concourse.mybir as mybir
import concourse.tile as tile
from concourse import bacc
from concourse._compat import with_exitstack
from concourse.bass_utils import run_bass_kernel_spmd

B, N, C = 16, 1024, 512
NUM_HEADS = 8
SCALE = (C // NUM_HEADS) ** -0.5  # 0.125
CORES = 8
BPC = B // CORES  # batches per core

P = 128          # partitions
NT = N // P      # 8 query tiles
CCH = C // P     # 4 contraction chunks over channels
MH = N // 512    # 2 query halves
MCH = N // P     # 8 key chunks
JT = C // P      # 4 output-column blocks per half

F32 = mybir.dt.float32
F32R = mybir.dt.float32r

USE_F32R = os.environ.get("KERNEL_F32R", "1") == "1"
WARMUP_MMS = int(os.environ.get("KERNEL_WARMUP", "50"))


def _r(ap):
    return ap.bitcast(F32R) if USE_F32R else ap


@with_exitstack
def _core_kernel(ctx: ExitStack, tc: tile.TileContext,
                 domt_d, reft_d, ref_d, wt_d, bias_d, eye_d, out_d):
    nc = tc.nc

    consts = ctx.enter_context(tc.tile_pool(name="consts", bufs=1))
    identity = consts.tile([P, P], F32)
    nc.sync.dma_start(_r(identity[:]), _r(eye_d[:, :]))

    ps_S = ctx.enter_context(tc.tile_pool(name="ps_s", bufs=3, space="PSUM"))
    ps_X = ctx.enter_context(tc.tile_pool(name="ps_x", bufs=3, space="PSUM"))
    ps_R = ctx.enter_context(tc.tile_pool(name="ps_r", bufs=1, space="PSUM"))
    ps_T = ctx.enter_context(tc.tile_pool(name="ps_t", bufs=1, space="PSUM"))

    # PE warmup: dependency-free matmuls on memset zeros while the first
    # input DMAs stream, so the HAM clock gate reaches full rate before
    # real work arrives.
    if WARMUP_MMS:
        zsrc = consts.tile([P, 640], F32)
        nc.vector.memset(zsrc[:], 0.0)
        zr = consts.tile([P, 640], F32)
        nc.vector.tensor_copy(_r(zr[:]), zsrc[:])
        warm_ps = ps_X.tile([P, 512], F32, tag="ps_x")
        for i in range(WARMUP_MMS):
            nc.tensor.matmul(warm_ps[:], _r(zr[:, :P]), _r(zr[:, P:640]),
                             start=True, stop=True)

    p_ref = ctx.enter_context(tc.tile_pool(name="ref", bufs=2))
    p_domT = ctx.enter_context(tc.tile_pool(name="domT", bufs=2))
    p_refT = ctx.enter_context(tc.tile_pool(name="refT", bufs=2))
    p_Pt = ctx.enter_context(tc.tile_pool(name="probsT", bufs=3))
    p_x = ctx.enter_context(tc.tile_pool(name="x", bufs=8))
    p_out = ctx.enter_context(tc.tile_pool(name="out", bufs=4))
    p_stats = ctx.enter_context(tc.tile_pool(name="stats", bufs=4))

    # ---- pre-emit every input DMA so the rings stream continuously ----
    def load_T(sb, dr, b, eng, k_granular=False):
        # [C, N] host-pretransposed tensor: chunk k lands at cols
        # [k*N, (k+1)*N); within a chunk, key/query index is the column.
        for h in range(MH):
            if k_granular:
                for k in range(CCH):
                    eng.dma_start(
                        _r(sb[:, k * N + h * 512: k * N + (h + 1) * 512]),
                        _r(dr[b, k * P:(k + 1) * P, h * 512:(h + 1) * 512]),
                    )
            else:
                eng.dma_start(
                    _r(sb[:, :].rearrange("p (k n) -> p k n", k=CCH)
                       [:, :, h * 512:(h + 1) * 512]),
                    _r(dr[b, :, h * 512:(h + 1) * 512]
                       .rearrange("(k p) c -> p k c", p=P)),
                )

    def load_nat(sb, dr, b, eng):
        # tile col block t holds rows [128t, 128(t+1)) of the [N, C] matrix
        eng.dma_start(
            _r(sb[:, :].rearrange("p (t c) -> p t c", t=NT)),
            _r(dr[b].rearrange("(t p) c -> p t c", p=P)),
        )

    domT_sbs = [p_domT.tile([P, CCH * N], F32, tag="domT", name=f"domT_sb{i}")
                for i in range(BPC)]
    refT_sbs = [p_refT.tile([P, CCH * N], F32, tag="refT", name=f"refT_sb{i}")
                for i in range(BPC)]
    ref_sbs = [p_ref.tile([P, NT * C], F32, tag="ref", name=f"ref_sb{i}")
               for i in range(BPC)]

    # Q1 (sync): eye, refT0, ref0, refT1, ref1
    load_T(refT_sbs[0], reft_d, 0, nc.sync, k_granular=True)
    load_nat(ref_sbs[0], ref_d, 0, nc.sync)
    # Q0 (gpsimd): domT0, wt, domT1, bias
    load_T(domT_sbs[0], domt_d, 0, nc.gpsimd, k_granular=True)
    wt_sb = consts.tile([P, CCH * C], F32)
    nc.gpsimd.dma_start(
        _r(wt_sb[:, :].rearrange("p (q c) -> p q c", q=CCH)),
        _r(wt_d.rearrange("(q p) c -> p q c", p=P)),
    )
    if BPC > 1:
        load_T(refT_sbs[1], reft_d, 1, nc.sync)
        load_nat(ref_sbs[1], ref_d, 1, nc.sync)
        load_T(domT_sbs[1], domt_d, 1, nc.gpsimd)
    bias_sb = consts.tile([P, C], F32)
    nc.gpsimd.dma_start(bias_sb[:], bias_d.partition_broadcast(P))

    # ones column for rowsum matmuls; rs staging tile for the recip bridge
    ones_f = consts.tile([P, 1], F32)
    nc.vector.memset(ones_f[:], 1.0)
    ones_r = consts.tile([P, 1], F32)
    nc.vector.tensor_copy(_r(ones_r[:]), ones_f[:])
    rs_sb = consts.tile([P, 512], F32)
    nc.vector.memset(rs_sb[:], 0.0)

    for b in range(BPC):
        domT_sb = domT_sbs[b]
        refT_sb = refT_sbs[b]
        ref_sb = ref_sbs[b]

        out_v = out_d[b].rearrange("(n2 two) j -> two n2 j", two=2)

        Pt_sbs = {}
        ps_rs = {}
        recip_sbs = {}
        x_tiles = []

        def mm1_group(h, mi):
            # S^T tile [key(mi) 128, query 512] -> exp into Pt
            if mi == 0:
                Pt_sbs[h] = p_Pt.tile([P, MCH * 512], F32, tag="probsT",
                                      name=f"Pt_sb{b}_{h}")
                ps_rs[h] = ps_R.tile([1, 512], F32, tag="ps_r",
                                     name=f"ps_r{b}_{h}")
            Pt_sb = Pt_sbs[h]
            ps_s = ps_S.tile([P, 512], F32, tag="ps_s",
                             name=f"ps_s{b}_{h}_{mi}")
            for k in range(CCH):
                nc.tensor.matmul(
                    ps_s[:],
                    _r(refT_sb[:, k * N + mi * P: k * N + (mi + 1) * P]),
                    _r(domT_sb[:, k * N + h * 512: k * N + (h + 1) * 512]),
                    start=(k == 0), stop=(k == CCH - 1),
                )
            nc.scalar.activation(_r(Pt_sb[:, mi * 512:(mi + 1) * 512]),
                                 ps_s[:],
                                 mybir.ActivationFunctionType.Exp,
                                 scale=float(SCALE))

        def rowsum(h, mi):
            # ones-lhsT matmul: [1,512] column sums of the P^T tile,
            # accumulated over mi into ps_r
            nc.tensor.matmul(ps_rs[h][:], _r(ones_r[:]),
                             _r(Pt_sbs[h][:, mi * 512:(mi + 1) * 512]),
                             start=(mi == 0), stop=(mi == MCH - 1))

        def bridge(h):
            # [1,512] rowsums -> SBUF -> 4 PE transposes -> reciprocal on
            # the 4 populated columns -> [128,4] per-partition scales
            nc.vector.tensor_copy(rs_sb[0:1, :], ps_rs.pop(h)[:])
            ps_t = ps_T.tile([P, 512], F32, tag="ps_t", name=f"ps_t{b}_{h}")
            for t in range(4):
                nc.tensor.transpose(ps_t[:, t * P:(t + 1) * P],
                                    rs_sb[:, t * P:(t + 1) * P],
                                    identity[:])
            recip_sb = p_stats.tile([P, 4], F32, tag="recip",
                                    name=f"recip{b}_{h}")
            nc.vector.reciprocal(
                recip_sb[:, :],
                ps_t[:, :].rearrange("p (t c) -> p t c", t=4)[:, :, 0])
            recip_sbs[h] = recip_sb

        def mm2_group(h, nl):
            # x tile [query 128, C] = sum_mi Pt(mi)^T @ ref chunk
            Pt_sb = Pt_sbs[h]
            ps_x = ps_X.tile([P, C], F32, tag="ps_x",
                             name=f"ps_x{b}_{h}_{nl}")
            for mi in range(MCH):
                nc.tensor.matmul(
                    ps_x[:],
                    _r(Pt_sb[:, mi * 512 + nl * P: mi * 512 + (nl + 1) * P]),
                    _r(ref_sb[:, mi * C:(mi + 1) * C]),
                    start=(mi == 0), stop=(mi == MCH - 1),
                )
            return ps_x

        def evict_x(h, nl, ps_x):
            x_t = p_x.tile([P, C], F32, tag="x", name=f"x_t{b}_{h}_{nl}")
            nc.vector.tensor_scalar_mul(_r(x_t[:]), ps_x[:],
                                        recip_sbs[h][:, nl:nl + 1])
            x_tiles.append(x_t)

        def mm3_group(e, cb):
            # out rows (2*cp + e) = x_half_e^T @ wt + bias; evict + store
            # in halves so the final store chain pipelines
            ps_z = ps_X.tile([P, C], F32, tag="ps_x",
                             name=f"ps_z{b}_{e}_{cb}")
            for q in range(CCH):
                x_t = x_tiles[e * CCH + q]
                nc.tensor.matmul(
                    ps_z[:],
                    _r(x_t[:, cb * P:(cb + 1) * P]),
                    _r(wt_sb[:, q * C:(q + 1) * C]),
                    start=(q == 0), stop=(q == CCH - 1),
                )
            o_sb = p_out.tile([P, C], F32, tag="out",
                              name=f"o_sb{b}_{e}_{cb}")
            if b == BPC - 1 and e == 1 and cb == JT - 1:
                # final store: split across both queues to shorten the tail
                for piece, eng in ((0, nc.gpsimd), (1, nc.sync)):
                    sl = slice(piece * (C // 2), (piece + 1) * (C // 2))
                    nc.vector.tensor_add(o_sb[:, sl], ps_z[:, sl],
                                         bias_sb[:, sl])
                    eng.dma_start(out_v[e, cb * P:(cb + 1) * P, sl],
                                  o_sb[:, sl])
            else:
                nc.vector.tensor_add(o_sb[:], ps_z[:], bias_sb[:])
                eng = nc.gpsimd if (e * JT + cb) % 2 == 0 else nc.sync
                eng.dma_start(out_v[e, cb * P:(cb + 1) * P, :], o_sb[:])

        # mm1 half 0 (rowsums lag two mi groups behind the exp evictions)
        for mi in range(MCH):
            mm1_group(0, mi)
            if mi >= 2:
                rowsum(0, mi - 2)
        rowsum(0, MCH - 2)
        rowsum(0, MCH - 1)
        # mm1 half 1, with half 0's recip bridge tucked after the first
        # group so the PE never waits on the vector engine
        mm1_group(1, 0)
        bridge(0)
        for mi in range(1, MCH):
            mm1_group(1, mi)
            if mi >= 2:
                rowsum(1, mi - 2)
        # mm2 half 0 starts before half 1's last two rowsums: it only needs
        # half 0 data, giving the scalar exp chain slack to finish half 1
        ps_x0 = mm2_group(0, 0)
        rowsum(1, MCH - 2)
        rowsum(1, MCH - 1)
        bridge(1)
        evict_x(0, 0, ps_x0)
        for nl in range(1, 4):
            evict_x(0, nl, mm2_group(0, nl))
        for cb in range(JT):
            mm3_group(0, cb)
        for nl in range(4):
            evict_x(1, nl, mm2_group(1, nl))
        for cb in range(JT):
            mm3_group(1, cb)


_CACHED = {}


def _build():
    key = ("nc", USE_F32R, WARMUP_MMS)
    if key in _CACHED:
        return _CACHED[key]
    nc = bacc.Bacc("TRN2", target_bir_lowering=False, debug=False)
    domt_d = nc.dram_tensor("domt", [BPC, C, N], F32, kind="ExternalInput").ap()
    reft_d = nc.dram_tensor("reft", [BPC, C, N], F32, kind="ExternalInput").ap()
    ref_d = nc.dram_tensor("ref", [BPC, N, C], F32, kind="ExternalInput").ap()
    wt_d = nc.dram_tensor("wt", [C, C], F32, kind="ExternalInput").ap()
    bias_d = nc.dram_tensor("bias", [C], F32, kind="ExternalInput").ap()
    eye_d = nc.dram_tensor("eye", [P, P], F32, kind="ExternalInput").ap()
    out_d = nc.dram_tensor("out", [BPC, N, C], F32, kind="ExternalOutput").ap()

    with tile.TileContext(nc) as tc:
        _core_kernel(tc, domt_d, reft_d, ref_d, wt_d, bias_d, eye_d, out_d)
    nc.compile()
    _CACHED[key] = nc
    return nc


LAST_RESULTS = None


def kernel(dom, ref, proj_w, proj_b):
    global LAST_RESULTS
    dom = np.ascontiguousarray(np.asarray(dom, dtype=np.float32))
    ref = np.ascontiguousarray(np.asarray(ref, dtype=np.float32))
    wt = np.ascontiguousarray(np.asarray(proj_w, dtype=np.float32).T)
    bias = np.ascontiguousarray(np.asarray(proj_b, dtype=np.float32))
    eye = np.eye(P, dtype=np.float32)

    domt = np.ascontiguousarray(dom.transpose(0, 2, 1))
    reft = np.ascontiguousarray(ref.transpose(0, 2, 1))
    nc = _build()
    in_maps = [
        {
            "domt": domt[c * BPC:(c + 1) * BPC],
            "reft": reft[c * BPC:(c + 1) * BPC],
            "ref": ref[c * BPC:(c + 1) * BPC],
            "wt": wt,
            "bias": bias,
            "eye": eye,
        }
        for c in range(CORES)
    ]
    res = run_bass_kernel_spmd(nc, in_maps, list(range(CORES)))
    LAST_RESULTS = res
    if res.exec_time_ns is not None:
        print(f"HW exec time: {res.exec_time_ns} ns")
    return np.concatenate([r["out"] for r in res.results], axis=0)
